# revision 1
# baseline (speedup 1.0000x reference)
"""MoLoRA (mixture of LoRA experts with top-2 routing) Trainium2 Bass kernel.

Math (per token t, hidden H=640, experts E=5, rank R=8, router hidden 256):
  h      = silu(x @ W1 + b1)                 [T, 256]
  logits = h @ W2 + b2                       [T, 5]
  top-2 of softmax(logits), renormalized  == softmax over the top-2 logits:
     u1 = sigmoid(m1 - m2), u2 = sigmoid(m2 - m1)  (m1/m2 = top-2 logit values)
  dense weights w[t, e] = u1*[e==argmax1] + u2*[e==argmax2]  (0 elsewhere)
  low    = x @ Acat                          [T, 40]   (Acat[h,(e,r)] = A[e,h,r])
  delta  = (low * w_expanded) @ Bcat * 2.0   [T, 640]  (Bcat[(e,r),h] = Bm[e,r,h])
  out    = base_output + delta

Sharding: data-parallel over 8 NeuronCores; each core takes 4096 tokens
(= one batch row), router/LoRA params replicated.
"""

import numpy as np
from contextlib import ExitStack

import concourse.bass as bass
import concourse.tile as tile
from concourse import bacc
from concourse import mybir
from concourse.bass import ts
from concourse.masks import make_identity
from concourse.bass_utils import run_bass_kernel_spmd

F32 = mybir.dt.float32
F32R = mybir.dt.float32r
AF = mybir.ActivationFunctionType
ALU = mybir.AluOpType
AX = mybir.AxisListType

H = 640          # hidden
E = 5            # experts
R = 8            # lora rank
ER = E * R       # 40
RH = 256         # router hidden
HC = H // 128    # 5 h-chunks
RC = RH // 128   # 2 router-hidden chunks
SCALING = 16.0 / R
N_CORES = 8
T_CORE = 4096    # tokens per core (32768 / 8)
TT = 256         # token tile (2 halves of 128)

# float32r ("fast fp32" PE mode) knobs for the non-router-critical matmuls.
LOW_FAST = False     # x @ Acat
DELTA_FAST = False   # lw @ Bcat
MM1_FAST = False     # x @ W1 (router -- precision critical, top-2 flips)
PIPE_ORDER = "frmb"  # fbrm | frmb | inline (sim-tuned)


def _f(ap, fast):
    """Optionally view an fp32 AP as float32r for the PE."""
    return ap.bitcast(F32R) if fast else ap


def build_kernel(t_core=T_CORE, niter=1, timing_mode=False, passes=1):
    assert t_core % TT == 0
    ntiles = t_core // TT
    nc = bacc.Bacc()

    if timing_mode:
        # big tensors stay on-device (uninitialized DRAM) so per-call wall
        # time isn't dominated by the axon host transfer; HBM traffic is
        # identical to the real kernel.
        x_d = nc.dram_tensor("x_int", [t_core, H], F32)[:, :]
        base_d = nc.dram_tensor("base_int", [t_core, H], F32)[:, :]
        out_d = nc.dram_tensor("out_int", [t_core, H], F32)[:, :]
        dummy_d = nc.declare_dram_parameter("dummy_out", [1, 4], F32, isOutput=True)
    else:
        x_d = nc.declare_dram_parameter("x", [t_core, H], F32, isOutput=False)
        base_d = nc.declare_dram_parameter("base", [t_core, H], F32, isOutput=False)
        out_d = nc.declare_dram_parameter("out", [t_core, H], F32, isOutput=True)
        dummy_d = None
    w1_d = nc.declare_dram_parameter("W1", [H, RH], F32, isOutput=False)
    b1_d = nc.declare_dram_parameter("b1", [RH], F32, isOutput=False)
    w2_d = nc.declare_dram_parameter("W2", [RH, E], F32, isOutput=False)
    b2_d = nc.declare_dram_parameter("b2", [E], F32, isOutput=False)
    a_d = nc.declare_dram_parameter("A", [E, H, R], F32, isOutput=False)
    bm_d = nc.declare_dram_parameter("Bm", [E, R, H], F32, isOutput=False)

    with ExitStack() as ctx:
        tc = ctx.enter_context(tile.TileContext(nc))
        const = ctx.enter_context(tc.tile_pool(name="const", bufs=1))
        xin_p = ctx.enter_context(tc.tile_pool(name="xin", bufs=3))
        bout_p = ctx.enter_context(tc.tile_pool(name="bout", bufs=4))
        xt_p = ctx.enter_context(tc.tile_pool(name="xt", bufs=2))
        ht_p = ctx.enter_context(tc.tile_pool(name="ht", bufs=2))
        small_p = ctx.enter_context(tc.tile_pool(name="small", bufs=4))
        lw_p = ctx.enter_context(tc.tile_pool(name="lw", bufs=3))
        ps_xt = ctx.enter_context(tc.tile_pool(name="ps_xt", bufs=1, space="PSUM"))
        ps_rt = ctx.enter_context(tc.tile_pool(name="ps_rt", bufs=1, space="PSUM"))
        ps_low = ctx.enter_context(tc.tile_pool(name="ps_low", bufs=2, space="PSUM"))
        ps_wrt = ctx.enter_context(tc.tile_pool(name="ps_wrt", bufs=1, space="PSUM"))
        ps_dl = ctx.enter_context(tc.tile_pool(name="ps_dl", bufs=1, space="PSUM"))

        # ---- constants / replicated params ----
        ident = const.tile([128, 128], F32)
        make_identity(nc, ident)

        w1_sb = const.tile([128, HC, RH], F32)
        nc.gpsimd.dma_start(out=w1_sb, in_=w1_d.rearrange("(c p) m -> p c m", p=128))
        b1_sb = const.tile([128, RC], F32)
        nc.gpsimd.dma_start(out=b1_sb, in_=b1_d.rearrange("(c p) -> p c", p=128))
        w2_sb = const.tile([128, RC, E], F32)
        nc.gpsimd.dma_start(out=w2_sb, in_=w2_d.rearrange("(c p) e -> p c e", p=128))
        b2_sb = const.tile([1, E], F32)
        nc.gpsimd.dma_start(out=b2_sb, in_=b2_d[:].unsqueeze(0))
        ones_sb = const.tile([1, 128], F32)
        nc.vector.memset(ones_sb, 1.0)
        # LoRA params concatenated over (e, r): index m = e*R + r.
        acat_sb = const.tile([128, HC, E, R], F32)
        for e in range(E):
            for c in range(HC):
                nc.gpsimd.dma_start(
                    out=acat_sb[:, c, e, :],
                    in_=a_d[e, c * 128 : (c + 1) * 128, :],
                )
        bcat_sb = const.tile([ER, H], F32)
        for e in range(E):
            nc.gpsimd.dma_start(out=bcat_sb[e * R : (e + 1) * R, :], in_=bm_d[e, :, :])
        ones8_sb = const.tile([128, R], F32)
        nc.vector.memset(ones8_sb, 1.0)
        # f32r-rounded copies of the LoRA params (PE float32r mode needs
        # producers that round; ~12-bit mantissa is plenty for the delta path)
        acat_r = const.tile([128, HC, E, R], F32R)
        nc.vector.tensor_copy(out=acat_r, in_=acat_sb)
        bcat_r = const.tile([ER, H], F32R)
        nc.vector.tensor_copy(out=bcat_r, in_=bcat_sb)
        # NOTE: LoRA SCALING (=2.0) is folded into the lw multiply below.

        if dummy_d is not None:
            dnm = const.tile([1, 4], F32)
            nc.vector.memset(dnm, 1.0)
            nc.sync.dma_start(out=dummy_d[:, :], in_=dnm)

        loop_ctx = tc.For_i(0, niter, 1) if niter > 1 else None
        if loop_ctx is not None:
            ctx.enter_context(loop_ctx)

        def emit_front(i):
            """loads + xT transposes"""
            tok = i * TT
            x_nat = xin_p.tile([128, 2, H], F32)
            nc.sync.dma_start(
                out=x_nat,
                in_=x_d[tok : tok + TT, :].rearrange("(j p) h -> p j h", p=128),
            )
            bo = bout_p.tile([128, 2, H], F32)
            nc.sync.dma_start(
                out=bo,
                in_=base_d[tok : tok + TT, :].rearrange("(j p) h -> p j h", p=128),
            )

            # transpose x to H-on-partitions; two PSUM->SBUF copies: exact
            # fp32 for the router mm1, f32r-rounded for the LoRA matmuls
            xt_sb = xt_p.tile([128, HC, TT], F32)
            xt_r = xt_p.tile([128, HC, TT], F32R, tag="xt_r")
            for j in range(2):
                xtp = ps_xt.tile([128, HC, 128], F32, tag="xtp")
                for c in range(HC):
                    nc.tensor.transpose(
                        out=xtp[:, c, :],
                        in_=x_nat[:, j, ts(c, 128)],
                        identity=ident,
                    )
                nc.any.tensor_copy(out=xt_sb[:, :, ts(j, 128)], in_=xtp)
                nc.scalar.copy(out=xt_r[:, :, ts(j, 128)], in_=xtp)
            return {"bo": bo, "xt_sb": xt_sb, "xt_r": xt_r, "tok": tok}

        def emit_router(st):
            """mm1 -> silu -> (lowT interleaved) -> mm2 -> routing weights"""
            xt_sb, xt_r = st["xt_sb"], st["xt_r"]
            # router mm1: hT[rh, t] = (x @ W1)^T (fp32, flip-sensitive)
            h_ps = ps_rt.tile([128, RC, TT], F32, tag="rt")
            for j in range(2):
                for c2 in range(RC):
                    for c in range(HC):
                        nc.tensor.matmul(
                            out=h_ps[:, c2, ts(j, 128)],
                            lhsT=w1_sb[:, c, ts(c2, 128)],
                            rhs=xt_sb[:, c, ts(j, 128)],
                            start=(c == 0),
                            stop=(c == HC - 1),
                        )
            # silu(z) = z * sigmoid(z), z = h + b1 (sigmoid table is the
            # most accurate ACT path; avoids table-set switches)
            ht_sb = ht_p.tile([128, RC, TT], F32)
            sg_sb = ht_p.tile([128, RC, TT], F32, tag="sg")
            for c2 in range(RC):
                nc.scalar.activation(
                    out=sg_sb[:, c2, :],
                    in_=h_ps[:, c2, :],
                    func=AF.Sigmoid,
                    bias=b1_sb[:, c2 : c2 + 1],
                )
                nc.vector.scalar_tensor_tensor(
                    out=ht_sb[:, c2, :],
                    in0=h_ps[:, c2, :],
                    scalar=b1_sb[:, c2 : c2 + 1],
                    in1=sg_sb[:, c2, :],
                    op0=ALU.add,
                    op1=ALU.mult,
                )

            # lowT[(e,r), t] = (x @ Acat)^T  (f32r) -- emitted here so the
            # PE has work while ACT/DVE finish silu before mm2
            low_ps = ps_low.tile([ER, TT], F32, tag="low")
            for c in range(HC):
                nc.tensor.matmul(
                    out=low_ps,
                    lhsT=acat_r[:, c, :, :],
                    rhs=xt_r[:, c, :],
                    start=(c == 0),
                    stop=(c == HC - 1),
                )
            st["low_ps"] = low_ps

            # router mm2 (token-major logits) + b2 via K=1 ones matmul
            lg_full = ps_rt.tile([128, RC, TT], F32, tag="rt")
            lg_ps = lg_full[:, :, 0:E]
            for j in range(2):
                for c2 in range(RC):
                    nc.tensor.matmul(
                        out=lg_ps[:, j, :],
                        lhsT=ht_sb[:, c2, ts(j, 128)],
                        rhs=w2_sb[:, c2, :],
                        start=(c2 == 0),
                        stop=False,
                    )
                nc.tensor.matmul(
                    out=lg_ps[:, j, :],
                    lhsT=ones_sb,
                    rhs=b2_sb,
                    start=False,
                    stop=True,
                )

            # top-2 selection reduces (everything up to the sigmoid input)
            top1 = small_p.tile([128, 2, E], F32, tag="top1")
            top2 = small_p.tile([128, 2, E], F32, tag="top2")
            dlg = small_p.tile([128, 2], F32, tag="dlg")
            for j in range(2):
                lg = lg_ps[:, j, :]
                m1 = small_p.tile([128, 1], F32, tag="m1")
                nc.vector.tensor_reduce(out=m1, in_=lg, axis=AX.X, op=ALU.max)
                nc.vector.tensor_scalar(
                    out=top1[:, j, :], in0=lg, scalar1=m1, scalar2=None,
                    op0=ALU.is_equal,
                )
                masked = small_p.tile([128, E], F32, tag="masked")
                nc.vector.scalar_tensor_tensor(
                    out=masked, in0=top1[:, j, :], scalar=-1e30, in1=lg,
                    op0=ALU.mult, op1=ALU.add,
                )
                m2 = small_p.tile([128, 1], F32, tag="m2")
                nc.vector.tensor_reduce(out=m2, in_=masked, axis=AX.X, op=ALU.max)
                nc.vector.tensor_scalar(
                    out=top2[:, j, :], in0=masked, scalar1=m2, scalar2=None,
                    op0=ALU.is_equal,
                )
                nc.vector.tensor_scalar(
                    out=dlg[:, j : j + 1], in0=m2, scalar1=m1, scalar2=None,
                    op0=ALU.subtract,
                )
            st["top1"], st["top2"], st["dlg"] = top1, top2, dlg

        def emit_weights(st):
            """softmax-over-top2 weights + expansion (one step later, so the
            ACT sigmoid never head-of-line blocks the next tile's copies)"""
            top1, top2, dlg = st["top1"], st["top2"], st["dlg"]
            w_small = small_p.tile([128, 2, E], F32)
            w_full = small_p.tile([128, 2, ER], F32)
            u2 = small_p.tile([128, 2], F32, tag="u2")
            nc.scalar.activation(out=u2, in_=dlg, func=AF.Sigmoid)
            for j in range(2):
                u2j = u2[:, j : j + 1]
                u1 = small_p.tile([128, 1], F32, tag="u1")
                nc.vector.tensor_scalar(
                    out=u1, in0=u2j, scalar1=-1.0, scalar2=1.0,
                    op0=ALU.mult, op1=ALU.add,
                )
                wt2 = small_p.tile([128, E], F32, tag="wt2")
                nc.vector.tensor_scalar(
                    out=wt2, in0=top2[:, j, :], scalar1=u2j, scalar2=None,
                    op0=ALU.mult,
                )
                nc.vector.scalar_tensor_tensor(
                    out=w_small[:, j, :], in0=top1[:, j, :], scalar=u1, in1=wt2,
                    op0=ALU.mult, op1=ALU.add,
                )
                # expand w[t, e] -> w_exp[t, e*R+r] via ones8 * w_e
                for e in range(E):
                    nc.vector.tensor_scalar(
                        out=w_full[:, j, e * R : (e + 1) * R],
                        in0=ones8_sb,
                        scalar1=w_small[:, j, e : e + 1],
                        scalar2=None,
                        op0=ALU.mult,
                    )
            st["w_full"] = w_full

        def emit_m(st):
            """middle: wrT transpose + weighted-low (feeds delta later)"""
            wrt_ps = ps_wrt.tile([ER, 2, 128], F32, tag="wrt")
            for j in range(2):
                nc.tensor.transpose(
                    out=wrt_ps[:, j, :], in_=st["w_full"][:, j, :], identity=ident
                )
            wrt_sb = small_p.tile([ER, 2, 128], F32, tag="wrt_sb")
            nc.any.tensor_copy(out=wrt_sb, in_=wrt_ps)
            lw_sb = lw_p.tile([ER, TT], F32R)
            nc.vector.scalar_tensor_tensor(
                out=lw_sb,
                in0=st["low_ps"],
                scalar=float(SCALING),
                in1=wrt_sb.rearrange("p j t -> p (j t)"),
                op0=ALU.mult,
                op1=ALU.mult,
            )
            st["lw_sb"] = lw_sb

        def emit_b(st):
            """back half: delta matmuls, base add, store"""
            bo, lw_sb, tok = st["bo"], st["lw_sb"], st["tok"]
            # delta in two 320-wide chunks (each >=256 keeps f32r at full
            # rate; each fits one PSUM bank in its own tile)
            for j in range(2):
                dla = ps_dl.tile([128, 320], F32, tag="dla")
                dlb = ps_dl.tile([128, 320], F32, tag="dlb")
                nc.tensor.matmul(
                    out=dla, lhsT=lw_sb[:, ts(j, 128)], rhs=bcat_r[:, 0:320],
                    start=True, stop=True,
                )
                nc.tensor.matmul(
                    out=dlb, lhsT=lw_sb[:, ts(j, 128)], rhs=bcat_r[:, 320:H],
                    start=True, stop=True,
                )
                nc.vector.tensor_tensor(
                    out=bo[:, j, 0:320], in0=dla, in1=bo[:, j, 0:320], op=ALU.add
                )
                nc.vector.tensor_tensor(
                    out=bo[:, j, 320:H], in0=dlb, in1=bo[:, j, 320:H], op=ALU.add
                )
            # store on the SWDGE (gpsimd) queue: keeps the waiting store off
            # the SP HWDGE FIFO so it can't head-of-line block the next loads
            nc.gpsimd.dma_start(
                out=out_d[tok : tok + TT, :].rearrange("(j p) h -> p j h", p=128),
                in_=bo,
            )

        prev = None
        for p in range(passes):
            for i in range(ntiles):
                st = emit_front(i)
                emit_router(st)
                emit_weights(st)
                emit_m(st)
                if prev is not None:
                    emit_b(prev)
                prev = st
        emit_b(prev)

    return nc


_CACHE = {}


def _get_nc(t_core=T_CORE, niter=1, timing_mode=False, passes=1):
    key = (t_core, niter, timing_mode, passes, MM1_FAST, LOW_FAST, DELTA_FAST)
    if key not in _CACHE:
        nc = build_kernel(t_core, niter, timing_mode, passes)
        nc.finalize()
        _CACHE[key] = nc
    return _CACHE[key]


def kernel(x, base_output, W1, b1, W2, b2, A, Bm):
    x = np.ascontiguousarray(np.asarray(x), dtype=np.float32)
    base_output = np.ascontiguousarray(np.asarray(base_output), dtype=np.float32)
    W1 = np.ascontiguousarray(np.asarray(W1), dtype=np.float32)
    b1 = np.ascontiguousarray(np.asarray(b1), dtype=np.float32)
    W2 = np.ascontiguousarray(np.asarray(W2), dtype=np.float32)
    b2 = np.ascontiguousarray(np.asarray(b2), dtype=np.float32)
    A = np.ascontiguousarray(np.asarray(A), dtype=np.float32)
    Bm = np.ascontiguousarray(np.asarray(Bm), dtype=np.float32)

    B, S, _ = x.shape
    assert B * S == N_CORES * T_CORE
    xs = x.reshape(N_CORES, T_CORE, H)
    bs = base_output.reshape(N_CORES, T_CORE, H)

    nc = _get_nc()
    in_maps = [
        {
            "x": np.ascontiguousarray(xs[i]),
            "base": np.ascontiguousarray(bs[i]),
            "W1": W1, "b1": b1, "W2": W2, "b2": b2, "A": A, "Bm": Bm,
        }
        for i in range(N_CORES)
    ]
    res = run_bass_kernel_spmd(nc, in_maps, list(range(N_CORES))).results
    out = np.stack([res[i]["out"] for i in range(N_CORES)], axis=0)
    return out.reshape(B, S, H).astype(np.float32)



# revision 9
# speedup vs baseline: 1.0772x; 1.0772x over previous
"""MoLoRA (mixture of LoRA experts with top-2 routing) Trainium2 Bass kernel.

Math (per token t, hidden H=640, experts E=5, rank R=8, router hidden 256):
  h      = silu(x @ W1 + b1)                 [T, 256]
  logits = h @ W2 + b2                       [T, 5]
  top-2 of softmax(logits), renormalized  == softmax over the top-2 logits:
     w[t, e] = sigmoid(2*l_e - m1 - m2) * [l_e >= m2]   (m1/m2 = top-2 logits)
  low    = x @ Acat                          [T, 40]   (Acat[h,(e,r)] = A[e,h,r])
  delta  = (low * w_expanded) @ (Bcat * 2)   [T, 640]  (Bcat[(e,r),h] = Bm[e,r,h])
  out    = base_output + delta

All matmuls run in float32r (fast-fp32 PE mode, full rate at N>=256); the
x transposes stay exact fp32.  Sharding: data-parallel over 8 NeuronCores;
each core takes 4096 tokens, router/LoRA params replicated.
"""

import numpy as np
from contextlib import ExitStack

import concourse.bass as bass
import concourse.tile as tile
from concourse import bacc
from concourse import mybir
from concourse.bass import ts
from concourse.masks import make_identity
from concourse.bass_utils import run_bass_kernel_spmd

F32 = mybir.dt.float32
F32R = mybir.dt.float32r
BF16 = mybir.dt.bfloat16
AF = mybir.ActivationFunctionType
ALU = mybir.AluOpType
AX = mybir.AxisListType

H = 640          # hidden
E = 5            # experts
R = 8            # lora rank
ER = E * R       # 40
RH = 256         # router hidden
HC = H // 128    # 5 h-chunks
RC = RH // 128   # 2 router-hidden chunks
SCALING = 16.0 / R
N_CORES = 8
T_CORE = 4096    # tokens per core (32768 / 8)
TT = 256         # token tile (2 halves of 128)

# engine placement knobs (tuned on hw)
SILU_MODE = "act_pool"   # "dve": stt on DVE | "act_pool": ACT identity + Pool mult
ADD_MODE = "dve"         # "dve": all 4 adds on DVE | "split": j1 via ACT copy + Pool add


def build_kernel(t_core=T_CORE, niter=1, timing_mode=False):
    assert t_core % TT == 0
    ntiles = t_core // TT
    nc = bacc.Bacc()

    if timing_mode:
        # big tensors stay on-device (uninitialized DRAM) so per-call wall
        # time isn't dominated by the axon host transfer; HBM traffic is
        # identical to the real kernel.
        x_d = nc.dram_tensor("x_int", [t_core, H], F32)[:, :]
        base_d = nc.dram_tensor("base_int", [t_core, H], F32)[:, :]
        out_d = nc.dram_tensor("out_int", [t_core, H], F32)[:, :]
        dummy_d = nc.declare_dram_parameter("dummy_out", [1, 4], F32, isOutput=True)
    else:
        x_d = nc.declare_dram_parameter("x", [t_core, H], F32, isOutput=False)
        base_d = nc.declare_dram_parameter("base", [t_core, H], F32, isOutput=False)
        out_d = nc.declare_dram_parameter("out", [t_core, H], F32, isOutput=True)
        dummy_d = None
    w1_d = nc.declare_dram_parameter("W1", [H, RH], F32, isOutput=False)
    b1_d = nc.declare_dram_parameter("b1", [RH], F32, isOutput=False)
    w2_d = nc.declare_dram_parameter("W2", [RH, E], F32, isOutput=False)
    b2_d = nc.declare_dram_parameter("b2", [E], F32, isOutput=False)
    a_d = nc.declare_dram_parameter("A", [E, H, R], F32, isOutput=False)
    bm_d = nc.declare_dram_parameter("Bm", [E, R, H], F32, isOutput=False)

    with ExitStack() as ctx:
        tc = ctx.enter_context(tile.TileContext(nc))
        const = ctx.enter_context(tc.tile_pool(name="const", bufs=1))
        xin_p = ctx.enter_context(tc.tile_pool(name="xin", bufs=3))
        bout_p = ctx.enter_context(tc.tile_pool(name="bout", bufs=4))
        xt_p = ctx.enter_context(tc.tile_pool(name="xt", bufs=2))
        ht_p = ctx.enter_context(tc.tile_pool(name="ht", bufs=2))
        small_p = ctx.enter_context(tc.tile_pool(name="small", bufs=2))
        lw_p = ctx.enter_context(tc.tile_pool(name="lw", bufs=2))
        # PSUM budget is 8 banks of 2KB:
        #  ps_h   [128, 2, 256] f32                      -> 1 bank,  bufs=1
        #  ps_xt  [128, 5, 256] f32                      -> 3 banks, bufs=1
        #  ps_lo  [128, 266]  = low (40x256) + lg (2x5)  -> 1 bank,  bufs=1
        #  ps_wrt [40, 256]                              -> 1 bank,  bufs=1
        #  ps_dl  dla/dlb [128, 320]                     -> 2 banks
        ps_h = ctx.enter_context(tc.tile_pool(name="ps_h", bufs=1, space="PSUM"))
        ps_xt = ctx.enter_context(tc.tile_pool(name="ps_xt", bufs=1, space="PSUM"))
        ps_lo = ctx.enter_context(tc.tile_pool(name="ps_lo", bufs=1, space="PSUM"))
        ps_wrt = ctx.enter_context(tc.tile_pool(name="ps_wrt", bufs=1, space="PSUM"))
        ps_dl = ctx.enter_context(tc.tile_pool(name="ps_dl", bufs=1, space="PSUM"))

        # ---- constants / replicated params ----
        ident = const.tile([128, 128], F32)
        make_identity(nc, ident)
        ident_bf = const.tile([128, 128], BF16)
        nc.vector.tensor_copy(out=ident_bf, in_=ident)

        w1_sb = const.tile([128, HC, RH], F32)
        nc.gpsimd.dma_start(out=w1_sb, in_=w1_d.rearrange("(c p) m -> p c m", p=128))
        w1_r = const.tile([128, HC, RH], F32R)
        nc.vector.tensor_copy(out=w1_r, in_=w1_sb)
        b1_sb = const.tile([128, RC], F32)
        nc.gpsimd.dma_start(out=b1_sb, in_=b1_d.rearrange("(c p) -> p c", p=128))
        w2_sb = const.tile([128, RC, E], F32)
        nc.gpsimd.dma_start(out=w2_sb, in_=w2_d.rearrange("(c p) e -> p c e", p=128))
        w2_r = w2_sb
        b2_sb = const.tile([1, E], F32)
        nc.gpsimd.dma_start(out=b2_sb, in_=b2_d[:].unsqueeze(0))
        b2_r = b2_sb
        ones_f = const.tile([1, 128], F32)
        nc.vector.memset(ones_f, 1.0)
        ones_r = ones_f
        # LoRA params concatenated over (e, r): index m = e*R + r.
        acat_sb = const.tile([128, HC, E, R], F32)
        for e in range(E):
            for c in range(HC):
                nc.gpsimd.dma_start(
                    out=acat_sb[:, c, e, :],
                    in_=a_d[e, c * 128 : (c + 1) * 128, :],
                )
        acat_r = const.tile([128, HC, E, R], F32R)
        nc.vector.tensor_copy(out=acat_r, in_=acat_sb)
        bcat_sb = const.tile([ER, H], F32)
        for e in range(E):
            nc.gpsimd.dma_start(out=bcat_sb[e * R : (e + 1) * R, :], in_=bm_d[e, :, :])
        # LoRA SCALING (=2.0) folded into Bcat here.
        bcat_r = const.tile([ER, H], F32R)
        nc.vector.tensor_scalar(
            out=bcat_r, in0=bcat_sb, scalar1=float(SCALING), scalar2=None, op0=ALU.mult
        )

        if dummy_d is not None:
            dnm = const.tile([1, 4], F32)
            nc.vector.memset(dnm, 1.0)
            nc.sync.dma_start(out=dummy_d[:, :], in_=dnm)

        loop_ctx = tc.For_i(0, niter, 1) if niter > 1 else None
        if loop_ctx is not None:
            ctx.enter_context(loop_ctx)

        def emit_front(i):
            """loads + xT transposes (exact fp32) + f32r copy"""
            tok = i * TT
            x_nat = xin_p.tile([128, 2, H], F32)
            nc.sync.dma_start(
                out=x_nat,
                in_=x_d[tok : tok + TT, :].rearrange("(j p) h -> p j h", p=128),
            )
            bo = bout_p.tile([128, 2, H], F32)
            nc.scalar.dma_start(
                out=bo,
                in_=base_d[tok : tok + TT, :].rearrange("(j p) h -> p j h", p=128),
            )

            h_ps = ps_h.tile([128, RC, TT], F32, tag="h")
            xtp = ps_xt.tile([128, HC, TT], F32, tag="xtp")
            for j in range(2):
                for c in range(HC):
                    nc.tensor.transpose(
                        out=xtp[:, c, ts(j, 128)],
                        in_=x_nat[:, j, ts(c, 128)],
                        identity=ident,
                    )
            xt_r = xt_p.tile([128, HC, TT], F32R)
            nc.scalar.copy(out=xt_r, in_=xtp)
            return {"bo": bo, "xt_r": xt_r, "h_ps": h_ps, "tok": tok}

        def emit_router(st):
            """mm1 -> silu -> (low interleaved) -> mm2 -> top-2 reduce"""
            xt_r, h_ps = st["xt_r"], st["h_ps"]
            # router mm1: hT[rh, t] = (x @ W1)^T, f32r, N=256 full rate
            for c2 in range(RC):
                for c in range(HC):
                    nc.tensor.matmul(
                        out=h_ps[:, c2, :],
                        lhsT=w1_r[:, c, ts(c2, 128)],
                        rhs=xt_r[:, c, :],
                        start=(c == 0),
                        stop=(c == HC - 1),
                    )
            # silu(z) = z * sigmoid(z), z = h + b1
            sg_sb = ht_p.tile([128, RC, TT], F32, tag="sg")
            ht_sb = ht_p.tile([128, RC, TT], F32, tag="ht")
            if SILU_MODE == "act_pool":
                hb_sb = ht_p.tile([128, RC, TT], F32, tag="hb")
                for c2 in range(RC):
                    nc.scalar.activation(
                        out=sg_sb[:, c2, :], in_=h_ps[:, c2, :],
                        func=AF.Sigmoid, bias=b1_sb[:, c2 : c2 + 1],
                    )
                    nc.scalar.activation(
                        out=hb_sb[:, c2, :], in_=h_ps[:, c2, :],
                        func=AF.Identity, bias=b1_sb[:, c2 : c2 + 1],
                    )
                nc.gpsimd.tensor_tensor(out=ht_sb, in0=hb_sb, in1=sg_sb, op=ALU.mult)
            else:
                for c2 in range(RC):
                    nc.scalar.activation(
                        out=sg_sb[:, c2, :], in_=h_ps[:, c2, :],
                        func=AF.Sigmoid, bias=b1_sb[:, c2 : c2 + 1],
                    )
                    nc.vector.scalar_tensor_tensor(
                        out=ht_sb[:, c2, :], in0=h_ps[:, c2, :],
                        scalar=b1_sb[:, c2 : c2 + 1], in1=sg_sb[:, c2, :],
                        op0=ALU.add, op1=ALU.mult,
                    )

            # low + logits share one PSUM bank: [40, 0:256]=lowT, [:, 256:266]=lg
            lo = ps_lo.tile([128, 266], F32, tag="lo")
            low_ps = lo[0:ER, 0:TT]
            lg_ps = lo[:, TT : TT + 2 * E].rearrange("p (j e) -> p j e", j=2)
            # lowT[(e,r), t] = (x @ Acat)^T (f32r, N=256)
            for c in range(HC):
                nc.tensor.matmul(
                    out=low_ps,
                    lhsT=acat_r[:, c, :, :],
                    rhs=xt_r[:, c, :],
                    start=(c == 0),
                    stop=(c == HC - 1),
                )
            # router mm2 (token-major logits) + b2 via K=1 ones matmul
            for j in range(2):
                for c2 in range(RC):
                    nc.tensor.matmul(
                        out=lg_ps[:, j, :],
                        lhsT=ht_sb[:, c2, ts(j, 128)],
                        rhs=w2_r[:, c2, :],
                        start=(c2 == 0),
                        stop=False,
                    )
                nc.tensor.matmul(
                    out=lg_ps[:, j, :],
                    lhsT=ones_r,
                    rhs=b2_r,
                    start=False,
                    stop=True,
                )
            st["low_ps"], st["lg_ps"] = low_ps, lg_ps

        def emit_weights(st):
            """j-merged top-2 + renormalized weights:
            w[e] = sigmoid(2*lg_e - m1 - m2) * [lg_e >= m2], expanded over r."""
            lg = st["lg_ps"]
            m1 = small_p.tile([128, 2], F32, tag="m1")
            nc.vector.tensor_reduce(out=m1, in_=lg, axis=AX.X, op=ALU.max)
            mask1 = small_p.tile([128, 2, E], F32, tag="mask1")
            nc.vector.tensor_tensor(
                out=mask1, in0=lg, in1=m1.unsqueeze(2).to_broadcast((128, 2, E)),
                op=ALU.is_equal,
            )
            masked = small_p.tile([128, 2, E], F32, tag="masked")
            nc.vector.scalar_tensor_tensor(
                out=masked, in0=mask1, scalar=-1e30, in1=lg,
                op0=ALU.mult, op1=ALU.add,
            )
            m2 = small_p.tile([128, 2], F32, tag="m2")
            nc.vector.tensor_reduce(out=m2, in_=masked, axis=AX.X, op=ALU.max)
            s2 = small_p.tile([128, 2], F32, tag="s2")
            nc.vector.tensor_tensor(out=s2, in0=m1, in1=m2, op=ALU.add)
            argt = small_p.tile([128, 2, E], F32, tag="argt")
            nc.vector.scalar_tensor_tensor(
                out=argt, in0=lg, scalar=2.0,
                in1=s2.unsqueeze(2).to_broadcast((128, 2, E)),
                op0=ALU.mult, op1=ALU.subtract,
            )
            sig = small_p.tile([128, 2, E], F32, tag="sig")
            nc.scalar.activation(out=sig, in_=argt, func=AF.Sigmoid)
            mge = small_p.tile([128, 2, E], F32, tag="mge")
            nc.vector.tensor_tensor(
                out=mge, in0=lg, in1=m2.unsqueeze(2).to_broadcast((128, 2, E)),
                op=ALU.is_ge,
            )
            # fused weight + expansion over r: w_exp[t, j, e, r] = sig*mge
            w_exp = small_p.tile([128, 2, E, R], BF16, tag="w_exp")
            nc.vector.tensor_tensor(
                out=w_exp,
                in0=sig.unsqueeze(3).to_broadcast((128, 2, E, R)),
                in1=mge.unsqueeze(3).to_broadcast((128, 2, E, R)),
                op=ALU.mult,
            )
            st["w_exp"] = w_exp

        def emit_m(st):
            """wT transpose + weighted-low (feeds delta later)"""
            wrt_ps = ps_wrt.tile([ER, 2, 128], BF16, tag="wrt")
            for j in range(2):
                nc.tensor.transpose(
                    out=wrt_ps[:, j, :],
                    in_=st["w_exp"][:, j, :, :].rearrange("p e r -> p (e r)"),
                    identity=ident_bf,
                )
            wrt_sb = lw_p.tile([ER, 2, 128], BF16, tag="wrt_sb")
            nc.scalar.copy(out=wrt_sb, in_=wrt_ps)
            lw_sb = lw_p.tile([ER, TT], F32R)
            nc.vector.tensor_tensor(
                out=lw_sb,
                in0=st["low_ps"],
                in1=wrt_sb.rearrange("p j t -> p (j t)"),
                op=ALU.mult,
            )
            st["lw_sb"] = lw_sb

        def emit_b(st):
            """back half: delta matmuls (f32r, N=320), base add, store"""
            bo, lw_sb, tok = st["bo"], st["lw_sb"], st["tok"]
            for j in range(2):
                dla = ps_dl.tile([128, 320], F32, tag="dla")
                dlb = ps_dl.tile([128, 320], F32, tag="dlb")
                nc.tensor.matmul(
                    out=dla, lhsT=lw_sb[:, ts(j, 128)], rhs=bcat_r[:, 0:320],
                    start=True, stop=True,
                )
                nc.tensor.matmul(
                    out=dlb, lhsT=lw_sb[:, ts(j, 128)], rhs=bcat_r[:, 320:H],
                    start=True, stop=True,
                )
                if ADD_MODE == "split" and j == 1:
                    dcp = lw_p.tile([128, 2, 320], F32, tag="dcp")
                    nc.scalar.copy(out=dcp[:, 0, :], in_=dla)
                    nc.scalar.copy(out=dcp[:, 1, :], in_=dlb)
                    nc.gpsimd.tensor_tensor(
                        out=bo[:, j, 0:320], in0=dcp[:, 0, :], in1=bo[:, j, 0:320],
                        op=ALU.add,
                    )
                    nc.gpsimd.tensor_tensor(
                        out=bo[:, j, 320:H], in0=dcp[:, 1, :], in1=bo[:, j, 320:H],
                        op=ALU.add,
                    )
                else:
                    nc.vector.tensor_tensor(
                        out=bo[:, j, 0:320], in0=dla, in1=bo[:, j, 0:320], op=ALU.add
                    )
                    nc.vector.tensor_tensor(
                        out=bo[:, j, 320:H], in0=dlb, in1=bo[:, j, 320:H], op=ALU.add
                    )
            # store on the SWDGE (gpsimd) queue: keeps the waiting store off
            # the HWDGE FIFOs so it can't head-of-line block the next loads
            nc.gpsimd.dma_start(
                out=out_d[tok : tok + TT, :].rearrange("(j p) h -> p j h", p=128),
                in_=bo,
            )

        prev = None
        for i in range(ntiles):
            st = emit_front(i)
            emit_router(st)
            emit_weights(st)
            emit_m(st)
            if prev is not None:
                emit_b(prev)
            prev = st
        emit_b(prev)

    return nc


_CACHE = {}


def _get_nc(t_core=T_CORE, niter=1, timing_mode=False):
    key = (t_core, niter, timing_mode, SILU_MODE, ADD_MODE)
    if key not in _CACHE:
        nc = build_kernel(t_core, niter, timing_mode)
        nc.finalize()
        _CACHE[key] = nc
    return _CACHE[key]


def kernel(x, base_output, W1, b1, W2, b2, A, Bm):
    x = np.ascontiguousarray(np.asarray(x), dtype=np.float32)
    base_output = np.ascontiguousarray(np.asarray(base_output), dtype=np.float32)
    W1 = np.ascontiguousarray(np.asarray(W1), dtype=np.float32)
    b1 = np.ascontiguousarray(np.asarray(b1), dtype=np.float32)
    W2 = np.ascontiguousarray(np.asarray(W2), dtype=np.float32)
    b2 = np.ascontiguousarray(np.asarray(b2), dtype=np.float32)
    A = np.ascontiguousarray(np.asarray(A), dtype=np.float32)
    Bm = np.ascontiguousarray(np.asarray(Bm), dtype=np.float32)

    B, S, _ = x.shape
    assert B * S == N_CORES * T_CORE
    xs = x.reshape(N_CORES, T_CORE, H)
    bs = base_output.reshape(N_CORES, T_CORE, H)

    nc = _get_nc()
    in_maps = [
        {
            "x": np.ascontiguousarray(xs[i]),
            "base": np.ascontiguousarray(bs[i]),
            "W1": W1, "b1": b1, "W2": W2, "b2": b2, "A": A, "Bm": Bm,
        }
        for i in range(N_CORES)
    ]
    res = run_bass_kernel_spmd(nc, in_maps, list(range(N_CORES))).results
    out = np.stack([res[i]["out"] for i in range(N_CORES)], axis=0)
    return out.reshape(B, S, H).astype(np.float32)


# revision 13
# speedup vs baseline: 1.4092x; 1.3081x over previous
"""MoLoRA (mixture of LoRA experts with top-2 routing) Trainium2 Bass kernel.

Math (per token t, hidden H=640, experts E=5, rank R=8, router hidden 256):
  h      = silu(x @ W1 + b1)                 [T, 256]
  logits = h @ W2 + b2                       [T, 5]
  top-2 of softmax(logits), renormalized  == softmax over the top-2 logits:
     w[t, e] = sigmoid(2*l_e - m1 - m2) * [l_e >= m2]   (m1/m2 = top-2 logits)
  low    = x @ Acat                          [T, 40]   (Acat[h,(e,r)] = A[e,h,r])
  delta  = (low * w_expanded) @ (Bcat * 2)   [T, 640]  (Bcat[(e,r),h] = Bm[e,r,h])
  out    = base_output + delta

mm1/low run in float32r (fast-fp32 PE mode, full rate at N>=256) off exact
fp32 PE transposes of x; mm2 is exact fp32 (tiny); the delta path is bf16.
Tiles are 512 tokens to amortize per-instruction overhead.  Sharding:
data-parallel over 8 NeuronCores (4096 tokens each), params replicated.
"""

import numpy as np
from contextlib import ExitStack

import concourse.bass as bass
import concourse.tile as tile
from concourse import bacc
from concourse import mybir
from concourse.bass import ts
from concourse.masks import make_identity
from concourse.bass_utils import run_bass_kernel_spmd

F32 = mybir.dt.float32
F32R = mybir.dt.float32r
BF16 = mybir.dt.bfloat16
AF = mybir.ActivationFunctionType
ALU = mybir.AluOpType
AX = mybir.AxisListType

H = 640          # hidden
E = 5            # experts
R = 8            # lora rank
ER = E * R       # 40
RH = 256         # router hidden
HC = H // 128    # 5 h-chunks
RC = RH // 128   # 2 router-hidden chunks
SCALING = 16.0 / R
N_CORES = 8
T_CORE = 4096    # tokens per core (32768 / 8)
TT = 512         # token tile (4 j-halves of 128)
JJ = TT // 128   # 4

# delta is computed in 5 PSUM-bank-aligned 512-column chunks of the flat
# (j, h) output: chunk -> list of (j, h0, h1, dl_offset)
DELTA_CHUNKS = []
for _c5 in range(5):
    _g0, _g1 = _c5 * 512, (_c5 + 1) * 512
    _parts = []
    for _j in range(JJ):
        _a, _b = max(_g0, _j * H), min(_g1, (_j + 1) * H)
        if _a < _b:
            _parts.append((_j, _a - _j * H, _b - _j * H, _a - _g0))
    DELTA_CHUNKS.append(_parts)


def build_kernel(t_core=T_CORE, niter=1, timing_mode=False):
    assert t_core % TT == 0
    ntiles = t_core // TT
    nc = bacc.Bacc()

    if timing_mode:
        # big tensors stay on-device (uninitialized DRAM) so per-call wall
        # time isn't dominated by the axon host transfer; HBM traffic is
        # identical to the real kernel.
        x_d = nc.dram_tensor("x_int", [t_core, H], F32)[:, :]
        base_d = nc.dram_tensor("base_int", [t_core, H], F32)[:, :]
        out_d = nc.dram_tensor("out_int", [t_core, H], F32)[:, :]
        dummy_d = nc.declare_dram_parameter("dummy_out", [1, 4], F32, isOutput=True)
    else:
        x_d = nc.declare_dram_parameter("x", [t_core, H], F32, isOutput=False)
        base_d = nc.declare_dram_parameter("base", [t_core, H], F32, isOutput=False)
        out_d = nc.declare_dram_parameter("out", [t_core, H], F32, isOutput=True)
        dummy_d = None
    w1_d = nc.declare_dram_parameter("W1", [H, RH], F32, isOutput=False)
    b1_d = nc.declare_dram_parameter("b1", [RH], F32, isOutput=False)
    w2_d = nc.declare_dram_parameter("W2", [RH, E], F32, isOutput=False)
    b2_d = nc.declare_dram_parameter("b2", [E], F32, isOutput=False)
    a_d = nc.declare_dram_parameter("A", [E, H, R], F32, isOutput=False)
    bm_d = nc.declare_dram_parameter("Bm", [E, R, H], F32, isOutput=False)

    with ExitStack() as ctx:
        tc = ctx.enter_context(tile.TileContext(nc))
        const = ctx.enter_context(tc.tile_pool(name="const", bufs=1))
        xin_p = ctx.enter_context(tc.tile_pool(name="xin", bufs=3))
        bout_p = ctx.enter_context(tc.tile_pool(name="bout", bufs=3))
        xt_p = ctx.enter_context(tc.tile_pool(name="xt", bufs=3))
        ht_p = ctx.enter_context(tc.tile_pool(name="ht", bufs=2))
        small_p = ctx.enter_context(tc.tile_pool(name="small", bufs=2))
        lw_p = ctx.enter_context(tc.tile_pool(name="lw", bufs=2))
        # PSUM budget (8 banks of 2KB):
        #  ps_xtp [128, 512] f32 bufs=2                  -> 2 banks
        #  ps_h   [128, 2, 512] f32 bufs=1               -> 2 banks
        #  ps_lo  [40, 512] f32 bufs=1                   -> 1 bank
        #  ps_wl  [128, 512] f32 bufs=1 (lg + wrt bf16)  -> 1 bank
        #  ps_dl  [128, 512] f32 bufs=2                  -> 2 banks
        ps_xtp = ctx.enter_context(tc.tile_pool(name="ps_xtp", bufs=2, space="PSUM"))
        ps_h = ctx.enter_context(tc.tile_pool(name="ps_h", bufs=1, space="PSUM"))
        ps_lo = ctx.enter_context(tc.tile_pool(name="ps_lo", bufs=1, space="PSUM"))
        ps_wl = ctx.enter_context(tc.tile_pool(name="ps_wl", bufs=1, space="PSUM"))
        ps_dl = ctx.enter_context(tc.tile_pool(name="ps_dl", bufs=2, space="PSUM"))

        # ---- constants / replicated params ----
        ident = const.tile([128, 128], F32)
        make_identity(nc, ident)
        ident_bf = const.tile([128, 128], BF16)
        nc.vector.tensor_copy(out=ident_bf, in_=ident)

        w1_sb = const.tile([128, HC, RH], F32)
        nc.gpsimd.dma_start(out=w1_sb, in_=w1_d.rearrange("(c p) m -> p c m", p=128))
        w1_r = const.tile([128, HC, RH], F32R)
        nc.vector.tensor_copy(out=w1_r, in_=w1_sb)
        b1_sb = const.tile([128, RC], F32)
        nc.gpsimd.dma_start(out=b1_sb, in_=b1_d.rearrange("(c p) -> p c", p=128))
        w2_sb = const.tile([128, RC, E], F32)
        nc.gpsimd.dma_start(out=w2_sb, in_=w2_d.rearrange("(c p) e -> p c e", p=128))
        # b2 replicated to all partitions (added on DVE, not via PE)
        b2_rep = const.tile([128, E], F32)
        nc.gpsimd.dma_start(
            out=b2_rep, in_=b2_d[:].unsqueeze(0).to_broadcast((128, E))
        )
        # LoRA params concatenated over (e, r): index m = e*R + r.
        acat_sb = const.tile([128, HC, E, R], F32)
        for e in range(E):
            for c in range(HC):
                nc.gpsimd.dma_start(
                    out=acat_sb[:, c, e, :],
                    in_=a_d[e, c * 128 : (c + 1) * 128, :],
                )
        acat_r = const.tile([128, HC, E, R], F32R)
        nc.vector.tensor_copy(out=acat_r, in_=acat_sb)
        bcat_sb = const.tile([ER, H], F32)
        for e in range(E):
            nc.gpsimd.dma_start(out=bcat_sb[e * R : (e + 1) * R, :], in_=bm_d[e, :, :])
        # LoRA SCALING (=2.0) folded into Bcat here; delta path is bf16.
        bcat_bf = const.tile([ER, H], BF16)
        nc.vector.tensor_scalar(
            out=bcat_bf, in0=bcat_sb, scalar1=float(SCALING), scalar2=None,
            op0=ALU.mult,
        )

        if dummy_d is not None:
            dnm = const.tile([1, 4], F32)
            nc.vector.memset(dnm, 1.0)
            nc.sync.dma_start(out=dummy_d[:, :], in_=dnm)

        loop_ctx = tc.For_i(0, niter, 1) if niter > 1 else None
        if loop_ctx is not None:
            ctx.enter_context(loop_ctx)

        def emit_front(i):
            """loads + xT transposes (exact fp32) + f32r copy, c-chunk rotated"""
            tok = i * TT
            x_nat = xin_p.tile([128, JJ, H], F32)
            nc.sync.dma_start(
                out=x_nat,
                in_=x_d[tok : tok + TT, :].rearrange("(j p) h -> p j h", p=128),
            )
            bo = bout_p.tile([128, JJ, H], F32)
            nc.scalar.dma_start(
                out=bo,
                in_=base_d[tok : tok + TT, :].rearrange("(j p) h -> p j h", p=128),
            )
            xt_r = xt_p.tile([128, HC, TT], F32R)
            for c in range(HC):
                xtp = ps_xtp.tile([128, TT], F32, tag="xtp")
                for tj in range(JJ):
                    nc.tensor.transpose(
                        out=xtp[:, ts(tj, 128)],
                        in_=x_nat[:, tj, ts(c, 128)],
                        identity=ident,
                    )
                nc.any.tensor_copy(out=xt_r[:, c, :], in_=xtp)
            return {"bo": bo, "xt_r": xt_r, "tok": tok}

        def emit_router(st):
            """mm1 -> silu -> (low interleaved) -> mm2"""
            xt_r = st["xt_r"]
            h_ps = ps_h.tile([128, RC, TT], F32, tag="h")
            for c2 in range(RC):
                for c in range(HC):
                    nc.tensor.matmul(
                        out=h_ps[:, c2, :],
                        lhsT=w1_r[:, c, ts(c2, 128)],
                        rhs=xt_r[:, c, :],
                        start=(c == 0),
                        stop=(c == HC - 1),
                    )
            # silu(z) = z * sigmoid(z), z = h + b1: ACT computes sigmoid and
            # z (identity+bias), Pool multiplies (keeps DVE free)
            sg_sb = ht_p.tile([128, RC, TT], F32, tag="sg")
            hb_sb = ht_p.tile([128, RC, TT], F32, tag="hb")
            ht_sb = ht_p.tile([128, RC, TT], F32, tag="ht")
            for c2 in range(RC):
                nc.scalar.activation(
                    out=sg_sb[:, c2, :], in_=h_ps[:, c2, :],
                    func=AF.Sigmoid, bias=b1_sb[:, c2 : c2 + 1],
                )
                nc.scalar.activation(
                    out=hb_sb[:, c2, :], in_=h_ps[:, c2, :],
                    func=AF.Identity, bias=b1_sb[:, c2 : c2 + 1],
                )
            nc.gpsimd.tensor_tensor(out=ht_sb, in0=hb_sb, in1=sg_sb, op=ALU.mult)

            # lowT[(e,r), t] = (x @ Acat)^T (f32r, N=512)
            low_ps = ps_lo.tile([ER, TT], F32, tag="lo")
            for c in range(HC):
                nc.tensor.matmul(
                    out=low_ps,
                    lhsT=acat_r[:, c, :, :],
                    rhs=xt_r[:, c, :],
                    start=(c == 0),
                    stop=(c == HC - 1),
                )
            # wl bank: lg f32 in [:, 0:20], wrt bf16 in f32-cols [128:384]
            wl = ps_wl.tile([128, 512], F32, tag="wl")
            lg_ps = wl[:, 0 : JJ * E].rearrange("p (j e) -> p j e", j=JJ)
            # router mm2 (token-major logits), exact fp32, b2 added on DVE
            for j in range(JJ):
                for c2 in range(RC):
                    nc.tensor.matmul(
                        out=lg_ps[:, j, :],
                        lhsT=ht_sb[:, c2, ts(j, 128)],
                        rhs=w2_sb[:, c2, :],
                        start=(c2 == 0),
                        stop=(c2 == RC - 1),
                    )
            st["low_ps"], st["wl"], st["lg_ps"] = low_ps, wl, lg_ps

        def emit_weights(st):
            """j-merged top-2 + renormalized weights:
            w[e] = sigmoid(2*lg_e - m1 - m2) * [lg_e >= m2], expanded over r."""
            lg = small_p.tile([128, JJ, E], F32, tag="lg")
            nc.vector.tensor_tensor(
                out=lg, in0=st["lg_ps"],
                in1=b2_rep.unsqueeze(1).to_broadcast((128, JJ, E)), op=ALU.add,
            )
            m1 = small_p.tile([128, JJ], F32, tag="m1")
            nc.vector.tensor_reduce(out=m1, in_=lg, axis=AX.X, op=ALU.max)
            mask1 = small_p.tile([128, JJ, E], F32, tag="mask1")
            nc.vector.tensor_tensor(
                out=mask1, in0=lg, in1=m1.unsqueeze(2).to_broadcast((128, JJ, E)),
                op=ALU.is_equal,
            )
            masked = small_p.tile([128, JJ, E], F32, tag="masked")
            nc.vector.scalar_tensor_tensor(
                out=masked, in0=mask1, scalar=-1e30, in1=lg,
                op0=ALU.mult, op1=ALU.add,
            )
            m2 = small_p.tile([128, JJ], F32, tag="m2")
            nc.vector.tensor_reduce(out=m2, in_=masked, axis=AX.X, op=ALU.max)
            s2 = small_p.tile([128, JJ], F32, tag="s2")
            nc.gpsimd.tensor_tensor(out=s2, in0=m1[:, :], in1=m2[:, :], op=ALU.add)
            argt = small_p.tile([128, JJ, E], F32, tag="argt")
            nc.vector.scalar_tensor_tensor(
                out=argt, in0=lg, scalar=2.0,
                in1=s2.unsqueeze(2).to_broadcast((128, JJ, E)),
                op0=ALU.mult, op1=ALU.subtract,
            )
            sig = small_p.tile([128, JJ, E], F32, tag="sig")
            nc.scalar.activation(out=sig, in_=argt, func=AF.Sigmoid)
            mge = small_p.tile([128, JJ, E], F32, tag="mge")
            nc.vector.tensor_tensor(
                out=mge, in0=lg, in1=m2.unsqueeze(2).to_broadcast((128, JJ, E)),
                op=ALU.is_ge,
            )
            # fused weight + expansion over r: w_exp[t, j, e, r] = sig*mge
            w_exp = small_p.tile([128, JJ, E, R], BF16, tag="w_exp")
            nc.vector.tensor_tensor(
                out=w_exp,
                in0=sig.unsqueeze(3).to_broadcast((128, JJ, E, R)),
                in1=mge.unsqueeze(3).to_broadcast((128, JJ, E, R)),
                op=ALU.mult,
            )
            st["w_exp"] = w_exp

        def emit_m(st):
            """wT transpose (into wl bank, bf16) + weighted-low"""
            # bf16 view of wl f32-cols [128:384] = 512 bf16 cols
            wrt_ps = st["wl"][:, 128:384].bitcast(BF16)[0:ER, :].rearrange(
                "p (j t) -> p j t", j=JJ
            )
            for j in range(JJ):
                nc.tensor.transpose(
                    out=wrt_ps[:, j, :],
                    in_=st["w_exp"][:, j, :, :].rearrange("p e r -> p (e r)"),
                    identity=ident_bf,
                )
            wrt_sb = lw_p.tile([ER, JJ, 128], BF16, tag="wrt_sb")
            nc.scalar.copy(out=wrt_sb, in_=wrt_ps)
            lw_sb = lw_p.tile([ER, TT], BF16)
            nc.vector.tensor_tensor(
                out=lw_sb,
                in0=st["low_ps"],
                in1=wrt_sb.rearrange("p j t -> p (j t)"),
                op=ALU.mult,
            )
            st["lw_sb"] = lw_sb

        def emit_b(st):
            """back half: delta matmuls (bf16) in 5 bank-aligned chunks of the
            flat (j, h) output, fused PSUM+base adds, store"""
            bo, lw_sb, tok = st["bo"], st["lw_sb"], st["tok"]
            bo_flat = bo.rearrange("p j h -> p (j h)")
            for c5, parts in enumerate(DELTA_CHUNKS):
                dl = ps_dl.tile([128, 512], F32, tag="dl")
                for j, h0, h1, off in parts:
                    nc.tensor.matmul(
                        out=dl[:, off : off + (h1 - h0)],
                        lhsT=lw_sb[:, ts(j, 128)],
                        rhs=bcat_bf[:, h0:h1],
                        start=True, stop=True,
                    )
                nc.vector.tensor_tensor(
                    out=bo_flat[:, c5 * 512 : (c5 + 1) * 512],
                    in0=dl,
                    in1=bo_flat[:, c5 * 512 : (c5 + 1) * 512],
                    op=ALU.add,
                )
            # store on the SWDGE (gpsimd) queue: keeps the waiting store off
            # the HWDGE FIFOs so it can't head-of-line block the next loads
            nc.gpsimd.dma_start(
                out=out_d[tok : tok + TT, :].rearrange("(j p) h -> p j h", p=128),
                in_=bo,
            )

        prev = None
        for i in range(ntiles):
            st = emit_front(i)
            emit_router(st)
            if prev is not None:
                emit_b(prev)
            emit_weights(st)
            emit_m(st)
            prev = st
        emit_b(prev)

    return nc


_CACHE = {}


def _get_nc(t_core=T_CORE, niter=1, timing_mode=False):
    key = (t_core, niter, timing_mode)
    if key not in _CACHE:
        nc = build_kernel(t_core, niter, timing_mode)
        nc.finalize()
        _CACHE[key] = nc
    return _CACHE[key]


def kernel(x, base_output, W1, b1, W2, b2, A, Bm):
    x = np.ascontiguousarray(np.asarray(x), dtype=np.float32)
    base_output = np.ascontiguousarray(np.asarray(base_output), dtype=np.float32)
    W1 = np.ascontiguousarray(np.asarray(W1), dtype=np.float32)
    b1 = np.ascontiguousarray(np.asarray(b1), dtype=np.float32)
    W2 = np.ascontiguousarray(np.asarray(W2), dtype=np.float32)
    b2 = np.ascontiguousarray(np.asarray(b2), dtype=np.float32)
    A = np.ascontiguousarray(np.asarray(A), dtype=np.float32)
    Bm = np.ascontiguousarray(np.asarray(Bm), dtype=np.float32)

    B, S, _ = x.shape
    assert B * S == N_CORES * T_CORE
    xs = x.reshape(N_CORES, T_CORE, H)
    bs = base_output.reshape(N_CORES, T_CORE, H)

    nc = _get_nc()
    in_maps = [
        {
            "x": np.ascontiguousarray(xs[i]),
            "base": np.ascontiguousarray(bs[i]),
            "W1": W1, "b1": b1, "W2": W2, "b2": b2, "A": A, "Bm": Bm,
        }
        for i in range(N_CORES)
    ]
    res = run_bass_kernel_spmd(nc, in_maps, list(range(N_CORES))).results
    out = np.stack([res[i]["out"] for i in range(N_CORES)], axis=0)
    return out.reshape(B, S, H).astype(np.float32)


# revision 14
# speedup vs baseline: 1.5387x; 1.0919x over previous
"""MoLoRA (mixture of LoRA experts with top-2 routing) Trainium2 Bass kernel.

Math (per token t, hidden H=640, experts E=5, rank R=8, router hidden 256):
  h      = silu(x @ W1 + b1)                 [T, 256]
  logits = h @ W2 + b2                       [T, 5]
  top-2 of softmax(logits), renormalized  == softmax over the top-2 logits:
     w[t, e] = sigmoid(2*l_e - m1 - m2) * [l_e >= m2]   (m1/m2 = top-2 logits)
  low    = x @ Acat                          [T, 40]   (Acat[h,(e,r)] = A[e,h,r])
  delta  = (low * w_expanded) @ (Bcat * 2)   [T, 640]  (Bcat[(e,r),h] = Bm[e,r,h])
  out    = base_output + delta

mm1/low run in float32r (fast-fp32 PE mode, full rate at N>=256) off exact
fp32 PE transposes of x; mm2 is exact fp32 (tiny); the delta path is bf16.
Tiles are 512 tokens to amortize per-instruction overhead.  Sharding:
data-parallel over 8 NeuronCores (4096 tokens each), params replicated.
"""

import numpy as np
from contextlib import ExitStack

import concourse.bass as bass
import concourse.tile as tile
from concourse import bacc
from concourse import mybir
from concourse.bass import ts
from concourse.masks import make_identity
from concourse.bass_utils import run_bass_kernel_spmd

F32 = mybir.dt.float32
F32R = mybir.dt.float32r
BF16 = mybir.dt.bfloat16
AF = mybir.ActivationFunctionType
ALU = mybir.AluOpType
AX = mybir.AxisListType

H = 640          # hidden
E = 5            # experts
R = 8            # lora rank
ER = E * R       # 40
RH = 256         # router hidden
HC = H // 128    # 5 h-chunks
RC = RH // 128   # 2 router-hidden chunks
SCALING = 16.0 / R
N_CORES = 8
T_CORE = 4096    # tokens per core (32768 / 8)
TT = 512         # token tile (4 j-halves of 128)
JJ = TT // 128   # 4

# delta is computed in 5 PSUM-bank-aligned 512-column chunks of the flat
# (j, h) output: chunk -> list of (j, h0, h1, dl_offset)
DELTA_CHUNKS = []
for _c5 in range(5):
    _g0, _g1 = _c5 * 512, (_c5 + 1) * 512
    _parts = []
    for _j in range(JJ):
        _a, _b = max(_g0, _j * H), min(_g1, (_j + 1) * H)
        if _a < _b:
            _parts.append((_j, _a - _j * H, _b - _j * H, _a - _g0))
    DELTA_CHUNKS.append(_parts)


def build_kernel(t_core=T_CORE, niter=1, timing_mode=False):
    assert t_core % TT == 0
    ntiles = t_core // TT
    nc = bacc.Bacc()

    if timing_mode:
        # big tensors stay on-device (uninitialized DRAM) so per-call wall
        # time isn't dominated by the axon host transfer; HBM traffic is
        # identical to the real kernel.
        x_d = nc.dram_tensor("x_int", [t_core, H], F32)[:, :]
        base_d = nc.dram_tensor("base_int", [t_core, H], F32)[:, :]
        out_d = nc.dram_tensor("out_int", [t_core, H], F32)[:, :]
        dummy_d = nc.declare_dram_parameter("dummy_out", [1, 4], F32, isOutput=True)
    else:
        x_d = nc.declare_dram_parameter("x", [t_core, H], F32, isOutput=False)
        base_d = nc.declare_dram_parameter("base", [t_core, H], F32, isOutput=False)
        out_d = nc.declare_dram_parameter("out", [t_core, H], F32, isOutput=True)
        dummy_d = None
    w1_d = nc.declare_dram_parameter("W1", [H, RH], F32, isOutput=False)
    b1_d = nc.declare_dram_parameter("b1", [RH], F32, isOutput=False)
    w2_d = nc.declare_dram_parameter("W2", [RH, E], F32, isOutput=False)
    b2_d = nc.declare_dram_parameter("b2", [E], F32, isOutput=False)
    a_d = nc.declare_dram_parameter("A", [E, H, R], F32, isOutput=False)
    bm_d = nc.declare_dram_parameter("Bm", [E, R, H], F32, isOutput=False)

    with ExitStack() as ctx:
        tc = ctx.enter_context(tile.TileContext(nc))
        const = ctx.enter_context(tc.tile_pool(name="const", bufs=1))
        xin_p = ctx.enter_context(tc.tile_pool(name="xin", bufs=3))
        bout_p = ctx.enter_context(tc.tile_pool(name="bout", bufs=4))
        xt_p = ctx.enter_context(tc.tile_pool(name="xt", bufs=3))
        ht_p = ctx.enter_context(tc.tile_pool(name="ht", bufs=3))
        small_p = ctx.enter_context(tc.tile_pool(name="small", bufs=3))
        lw_p = ctx.enter_context(tc.tile_pool(name="lw", bufs=3))
        # PSUM budget (8 banks of 2KB):
        #  ps_xtp [128, 512] f32 bufs=2                  -> 2 banks
        #  ps_h   [128, 2, 512] f32 bufs=1               -> 2 banks
        #  ps_lo  [40, 512] f32 bufs=1                   -> 1 bank
        #  ps_wl  [128, 512] f32 bufs=1 (lg + wrt bf16)  -> 1 bank
        #  ps_dl  [128, 512] f32 bufs=2                  -> 2 banks
        ps_xtp = ctx.enter_context(tc.tile_pool(name="ps_xtp", bufs=2, space="PSUM"))
        ps_h = ctx.enter_context(tc.tile_pool(name="ps_h", bufs=1, space="PSUM"))
        ps_lo = ctx.enter_context(tc.tile_pool(name="ps_lo", bufs=1, space="PSUM"))
        ps_wl = ctx.enter_context(tc.tile_pool(name="ps_wl", bufs=1, space="PSUM"))
        ps_dl = ctx.enter_context(tc.tile_pool(name="ps_dl", bufs=2, space="PSUM"))

        # ---- constants / replicated params ----
        ident = const.tile([128, 128], F32)
        make_identity(nc, ident)
        ident_bf = const.tile([128, 128], BF16)
        nc.vector.tensor_copy(out=ident_bf, in_=ident)

        w1_sb = const.tile([128, HC, RH], F32)
        nc.gpsimd.dma_start(out=w1_sb, in_=w1_d.rearrange("(c p) m -> p c m", p=128))
        w1_r = const.tile([128, HC, RH], F32R)
        nc.vector.tensor_copy(out=w1_r, in_=w1_sb)
        b1_sb = const.tile([128, RC], F32)
        nc.gpsimd.dma_start(out=b1_sb, in_=b1_d.rearrange("(c p) -> p c", p=128))
        w2_sb = const.tile([128, RC, E], F32)
        nc.gpsimd.dma_start(out=w2_sb, in_=w2_d.rearrange("(c p) e -> p c e", p=128))
        # b2 replicated to all partitions (added on DVE, not via PE)
        b2_rep = const.tile([128, E], F32)
        nc.gpsimd.dma_start(
            out=b2_rep, in_=b2_d[:].unsqueeze(0).to_broadcast((128, E))
        )
        # LoRA params concatenated over (e, r): index m = e*R + r.
        acat_sb = const.tile([128, HC, E, R], F32)
        for e in range(E):
            for c in range(HC):
                nc.gpsimd.dma_start(
                    out=acat_sb[:, c, e, :],
                    in_=a_d[e, c * 128 : (c + 1) * 128, :],
                )
        acat_r = const.tile([128, HC, E, R], F32R)
        nc.vector.tensor_copy(out=acat_r, in_=acat_sb)
        bcat_sb = const.tile([ER, H], F32)
        for e in range(E):
            nc.gpsimd.dma_start(out=bcat_sb[e * R : (e + 1) * R, :], in_=bm_d[e, :, :])
        # LoRA SCALING (=2.0) folded into Bcat here; delta path is bf16.
        bcat_bf = const.tile([ER, H], BF16)
        nc.vector.tensor_scalar(
            out=bcat_bf, in0=bcat_sb, scalar1=float(SCALING), scalar2=None,
            op0=ALU.mult,
        )

        if dummy_d is not None:
            dnm = const.tile([1, 4], F32)
            nc.vector.memset(dnm, 1.0)
            nc.sync.dma_start(out=dummy_d[:, :], in_=dnm)

        loop_ctx = tc.For_i(0, niter, 1) if niter > 1 else None
        if loop_ctx is not None:
            ctx.enter_context(loop_ctx)

        def emit_front(i):
            """loads + xT transposes (exact fp32) + f32r copy, c-chunk rotated"""
            tok = i * TT
            x_nat = xin_p.tile([128, JJ, H], F32)
            nc.sync.dma_start(
                out=x_nat,
                in_=x_d[tok : tok + TT, :].rearrange("(j p) h -> p j h", p=128),
            )
            bo = bout_p.tile([128, JJ, H], F32)
            nc.scalar.dma_start(
                out=bo,
                in_=base_d[tok : tok + TT, :].rearrange("(j p) h -> p j h", p=128),
            )
            xt_r = xt_p.tile([128, HC, TT], F32R)
            for c in range(HC):
                xtp = ps_xtp.tile([128, TT], F32, tag="xtp")
                for tj in range(JJ):
                    nc.tensor.transpose(
                        out=xtp[:, ts(tj, 128)],
                        in_=x_nat[:, tj, ts(c, 128)],
                        identity=ident,
                    )
                nc.any.tensor_copy(out=xt_r[:, c, :], in_=xtp)
            return {"bo": bo, "xt_r": xt_r, "tok": tok}

        def emit_router(st):
            """mm1 -> silu -> (low interleaved) -> mm2"""
            xt_r = st["xt_r"]
            h_ps = ps_h.tile([128, RC, TT], F32, tag="h")
            for c2 in range(RC):
                for c in range(HC):
                    nc.tensor.matmul(
                        out=h_ps[:, c2, :],
                        lhsT=w1_r[:, c, ts(c2, 128)],
                        rhs=xt_r[:, c, :],
                        start=(c == 0),
                        stop=(c == HC - 1),
                    )
            # silu(z) = z * sigmoid(z), z = h + b1: ACT computes sigmoid and
            # z (identity+bias), Pool multiplies (keeps DVE free)
            sg_sb = ht_p.tile([128, RC, TT], F32, tag="sg")
            hb_sb = ht_p.tile([128, RC, TT], F32, tag="hb")
            ht_sb = ht_p.tile([128, RC, TT], F32, tag="ht")
            for c2 in range(RC):
                nc.scalar.activation(
                    out=sg_sb[:, c2, :], in_=h_ps[:, c2, :],
                    func=AF.Sigmoid, bias=b1_sb[:, c2 : c2 + 1],
                )
                nc.scalar.activation(
                    out=hb_sb[:, c2, :], in_=h_ps[:, c2, :],
                    func=AF.Identity, bias=b1_sb[:, c2 : c2 + 1],
                )
            nc.gpsimd.tensor_tensor(out=ht_sb, in0=hb_sb, in1=sg_sb, op=ALU.mult)

            # lowT[(e,r), t] = (x @ Acat)^T (f32r, N=512)
            low_ps = ps_lo.tile([ER, TT], F32, tag="lo")
            for c in range(HC):
                nc.tensor.matmul(
                    out=low_ps,
                    lhsT=acat_r[:, c, :, :],
                    rhs=xt_r[:, c, :],
                    start=(c == 0),
                    stop=(c == HC - 1),
                )
            # wl bank: lg f32 in [:, 0:20], wrt bf16 in f32-cols [128:384]
            wl = ps_wl.tile([128, 512], F32, tag="wl")
            lg_ps = wl[:, 0 : JJ * E].rearrange("p (j e) -> p j e", j=JJ)
            # router mm2 (token-major logits), exact fp32, b2 added on DVE
            for j in range(JJ):
                for c2 in range(RC):
                    nc.tensor.matmul(
                        out=lg_ps[:, j, :],
                        lhsT=ht_sb[:, c2, ts(j, 128)],
                        rhs=w2_sb[:, c2, :],
                        start=(c2 == 0),
                        stop=(c2 == RC - 1),
                    )
            st["low_ps"], st["wl"], st["lg_ps"] = low_ps, wl, lg_ps

        def emit_weights(st):
            """j-merged top-2 + renormalized weights:
            w[e] = sigmoid(2*lg_e - m1 - m2) * [lg_e >= m2], expanded over r."""
            lg = small_p.tile([128, JJ, E], F32, tag="lg")
            nc.vector.tensor_tensor(
                out=lg, in0=st["lg_ps"],
                in1=b2_rep.unsqueeze(1).to_broadcast((128, JJ, E)), op=ALU.add,
            )
            m1 = small_p.tile([128, JJ], F32, tag="m1")
            nc.vector.tensor_reduce(out=m1, in_=lg, axis=AX.X, op=ALU.max)
            mask1 = small_p.tile([128, JJ, E], F32, tag="mask1")
            nc.vector.tensor_tensor(
                out=mask1, in0=lg, in1=m1.unsqueeze(2).to_broadcast((128, JJ, E)),
                op=ALU.is_equal,
            )
            masked = small_p.tile([128, JJ, E], F32, tag="masked")
            nc.vector.scalar_tensor_tensor(
                out=masked, in0=mask1, scalar=-1e30, in1=lg,
                op0=ALU.mult, op1=ALU.add,
            )
            m2 = small_p.tile([128, JJ], F32, tag="m2")
            nc.vector.tensor_reduce(out=m2, in_=masked, axis=AX.X, op=ALU.max)
            s2 = small_p.tile([128, JJ], F32, tag="s2")
            nc.vector.tensor_tensor(out=s2, in0=m1, in1=m2, op=ALU.add)
            argt = small_p.tile([128, JJ, E], F32, tag="argt")
            nc.vector.scalar_tensor_tensor(
                out=argt, in0=lg, scalar=2.0,
                in1=s2.unsqueeze(2).to_broadcast((128, JJ, E)),
                op0=ALU.mult, op1=ALU.subtract,
            )
            sig = small_p.tile([128, JJ, E], F32, tag="sig")
            nc.scalar.activation(out=sig, in_=argt, func=AF.Sigmoid)
            mge = small_p.tile([128, JJ, E], F32, tag="mge")
            nc.vector.tensor_tensor(
                out=mge, in0=lg, in1=m2.unsqueeze(2).to_broadcast((128, JJ, E)),
                op=ALU.is_ge,
            )
            # fused weight + expansion over r: w_exp[t, j, e, r] = sig*mge
            w_exp = small_p.tile([128, JJ, E, R], BF16, tag="w_exp")
            nc.vector.tensor_tensor(
                out=w_exp,
                in0=sig.unsqueeze(3).to_broadcast((128, JJ, E, R)),
                in1=mge.unsqueeze(3).to_broadcast((128, JJ, E, R)),
                op=ALU.mult,
            )
            st["w_exp"] = w_exp

        def emit_m(st):
            """wT transpose (into wl bank, bf16) + weighted-low"""
            # bf16 view of wl f32-cols [128:384] = 512 bf16 cols
            wrt_ps = st["wl"][:, 128:384].bitcast(BF16)[0:ER, :].rearrange(
                "p (j t) -> p j t", j=JJ
            )
            for j in range(JJ):
                nc.tensor.transpose(
                    out=wrt_ps[:, j, :],
                    in_=st["w_exp"][:, j, :, :].rearrange("p e r -> p (e r)"),
                    identity=ident_bf,
                )
            wrt_sb = lw_p.tile([ER, JJ, 128], BF16, tag="wrt_sb")
            nc.scalar.copy(out=wrt_sb, in_=wrt_ps)
            lw_sb = lw_p.tile([ER, TT], BF16)
            nc.vector.tensor_tensor(
                out=lw_sb,
                in0=st["low_ps"],
                in1=wrt_sb.rearrange("p j t -> p (j t)"),
                op=ALU.mult,
            )
            st["lw_sb"] = lw_sb

        def emit_b(st):
            """back half: delta matmuls (bf16) in 5 bank-aligned chunks of the
            flat (j, h) output, fused PSUM+base adds, store"""
            bo, lw_sb, tok = st["bo"], st["lw_sb"], st["tok"]
            bo_flat = bo.rearrange("p j h -> p (j h)")
            for c5, parts in enumerate(DELTA_CHUNKS):
                dl = ps_dl.tile([128, 512], F32, tag="dl")
                for j, h0, h1, off in parts:
                    nc.tensor.matmul(
                        out=dl[:, off : off + (h1 - h0)],
                        lhsT=lw_sb[:, ts(j, 128)],
                        rhs=bcat_bf[:, h0:h1],
                        start=True, stop=True,
                    )
                nc.vector.tensor_tensor(
                    out=bo_flat[:, c5 * 512 : (c5 + 1) * 512],
                    in0=dl,
                    in1=bo_flat[:, c5 * 512 : (c5 + 1) * 512],
                    op=ALU.add,
                )
            # store on the SWDGE (gpsimd) queue: keeps the waiting store off
            # the HWDGE FIFOs so it can't head-of-line block the next loads
            nc.gpsimd.dma_start(
                out=out_d[tok : tok + TT, :].rearrange("(j p) h -> p j h", p=128),
                in_=bo,
            )

        prev = None
        for i in range(ntiles):
            st = emit_front(i)
            if prev is not None:
                emit_b(prev)
            emit_router(st)
            emit_weights(st)
            emit_m(st)
            prev = st
        emit_b(prev)

    return nc


_CACHE = {}


def _get_nc(t_core=T_CORE, niter=1, timing_mode=False):
    key = (t_core, niter, timing_mode)
    if key not in _CACHE:
        nc = build_kernel(t_core, niter, timing_mode)
        nc.finalize()
        _CACHE[key] = nc
    return _CACHE[key]


def kernel(x, base_output, W1, b1, W2, b2, A, Bm):
    x = np.ascontiguousarray(np.asarray(x), dtype=np.float32)
    base_output = np.ascontiguousarray(np.asarray(base_output), dtype=np.float32)
    W1 = np.ascontiguousarray(np.asarray(W1), dtype=np.float32)
    b1 = np.ascontiguousarray(np.asarray(b1), dtype=np.float32)
    W2 = np.ascontiguousarray(np.asarray(W2), dtype=np.float32)
    b2 = np.ascontiguousarray(np.asarray(b2), dtype=np.float32)
    A = np.ascontiguousarray(np.asarray(A), dtype=np.float32)
    Bm = np.ascontiguousarray(np.asarray(Bm), dtype=np.float32)

    B, S, _ = x.shape
    assert B * S == N_CORES * T_CORE
    xs = x.reshape(N_CORES, T_CORE, H)
    bs = base_output.reshape(N_CORES, T_CORE, H)

    nc = _get_nc()
    in_maps = [
        {
            "x": np.ascontiguousarray(xs[i]),
            "base": np.ascontiguousarray(bs[i]),
            "W1": W1, "b1": b1, "W2": W2, "b2": b2, "A": A, "Bm": Bm,
        }
        for i in range(N_CORES)
    ]
    res = run_bass_kernel_spmd(nc, in_maps, list(range(N_CORES))).results
    out = np.stack([res[i]["out"] for i in range(N_CORES)], axis=0)
    return out.reshape(B, S, H).astype(np.float32)


# revision 15
# speedup vs baseline: 1.6473x; 1.0706x over previous
"""MoLoRA (mixture of LoRA experts with top-2 routing) Trainium2 Bass kernel.

Math (per token t, hidden H=640, experts E=5, rank R=8, router hidden 256):
  h      = silu(x @ W1 + b1)                 [T, 256]
  logits = h @ W2 + b2                       [T, 5]
  top-2 of softmax(logits), renormalized  == softmax over the top-2 logits:
     w[t, e] = sigmoid(2*l_e - m1 - m2) * [l_e >= m2]   (m1/m2 = top-2 logits)
  low    = x @ Acat                          [T, 40]   (Acat[h,(e,r)] = A[e,h,r])
  delta  = (low * w_expanded) @ (Bcat * 2)   [T, 640]  (Bcat[(e,r),h] = Bm[e,r,h])
  out    = base_output + delta

mm1/low run in float32r (fast-fp32 PE mode, full rate at N>=256) off exact
fp32 PE transposes of x; mm2 is exact fp32 (tiny); the delta path is bf16.
Tiles are 512 tokens to amortize per-instruction overhead.  Sharding:
data-parallel over 8 NeuronCores (4096 tokens each), params replicated.
"""

import numpy as np
from contextlib import ExitStack

import concourse.bass as bass
import concourse.tile as tile
from concourse import bacc
from concourse import mybir
from concourse.bass import ts
from concourse.masks import make_identity
from concourse.bass_utils import run_bass_kernel_spmd

F32 = mybir.dt.float32
F32R = mybir.dt.float32r
BF16 = mybir.dt.bfloat16
AF = mybir.ActivationFunctionType
ALU = mybir.AluOpType
AX = mybir.AxisListType

H = 640          # hidden
E = 5            # experts
R = 8            # lora rank
ER = E * R       # 40
RH = 256         # router hidden
HC = H // 128    # 5 h-chunks
RC = RH // 128   # 2 router-hidden chunks
SCALING = 16.0 / R
N_CORES = 8
T_CORE = 4096    # tokens per core (32768 / 8)
TT = 512         # token tile (4 j-halves of 128)
JJ = TT // 128   # 4

# delta is computed in 5 PSUM-bank-aligned 512-column chunks of the flat
# (j, h) output: chunk -> list of (j, h0, h1, dl_offset)
DELTA_CHUNKS = []
for _c5 in range(5):
    _g0, _g1 = _c5 * 512, (_c5 + 1) * 512
    _parts = []
    for _j in range(JJ):
        _a, _b = max(_g0, _j * H), min(_g1, (_j + 1) * H)
        if _a < _b:
            _parts.append((_j, _a - _j * H, _b - _j * H, _a - _g0))
    DELTA_CHUNKS.append(_parts)


def build_kernel(t_core=T_CORE, niter=1, timing_mode=False):
    assert t_core % TT == 0
    ntiles = t_core // TT
    nc = bacc.Bacc()

    if timing_mode:
        # big tensors stay on-device (uninitialized DRAM) so per-call wall
        # time isn't dominated by the axon host transfer; HBM traffic is
        # identical to the real kernel.
        x_d = nc.dram_tensor("x_int", [t_core, H], F32)[:, :]
        base_d = nc.dram_tensor("base_int", [t_core, H], F32)[:, :]
        out_d = nc.dram_tensor("out_int", [t_core, H], F32)[:, :]
        dummy_d = nc.declare_dram_parameter("dummy_out", [1, 4], F32, isOutput=True)
    else:
        x_d = nc.declare_dram_parameter("x", [t_core, H], F32, isOutput=False)
        base_d = nc.declare_dram_parameter("base", [t_core, H], F32, isOutput=False)
        out_d = nc.declare_dram_parameter("out", [t_core, H], F32, isOutput=True)
        dummy_d = None
    w1_d = nc.declare_dram_parameter("W1", [H, RH], F32, isOutput=False)
    b1_d = nc.declare_dram_parameter("b1", [RH], F32, isOutput=False)
    w2_d = nc.declare_dram_parameter("W2", [RH, E], F32, isOutput=False)
    b2_d = nc.declare_dram_parameter("b2", [E], F32, isOutput=False)
    a_d = nc.declare_dram_parameter("A", [E, H, R], F32, isOutput=False)
    bm_d = nc.declare_dram_parameter("Bm", [E, R, H], F32, isOutput=False)

    with ExitStack() as ctx:
        tc = ctx.enter_context(tile.TileContext(nc))
        const = ctx.enter_context(tc.tile_pool(name="const", bufs=1))
        xin_p = ctx.enter_context(tc.tile_pool(name="xin", bufs=2))
        bout_p = ctx.enter_context(tc.tile_pool(name="bout", bufs=3))
        xt_p = ctx.enter_context(tc.tile_pool(name="xt", bufs=2))
        ht_p = ctx.enter_context(tc.tile_pool(name="ht", bufs=2))
        small_p = ctx.enter_context(tc.tile_pool(name="small", bufs=2))
        lw_p = ctx.enter_context(tc.tile_pool(name="lw", bufs=2))
        # PSUM budget (8 banks of 2KB):
        #  ps_xtp [128, 512] f32 bufs=2                  -> 2 banks
        #  ps_h   [128, 2, 512] f32 bufs=1               -> 2 banks
        #  ps_lo  [40, 512] f32 bufs=1                   -> 1 bank
        #  ps_wl  [128, 512] f32 bufs=1 (lg + wrt bf16)  -> 1 bank
        #  ps_dl  [128, 512] f32 bufs=2                  -> 2 banks
        ps_xtp = ctx.enter_context(tc.tile_pool(name="ps_xtp", bufs=2, space="PSUM"))
        ps_h = ctx.enter_context(tc.tile_pool(name="ps_h", bufs=1, space="PSUM"))
        ps_lo = ctx.enter_context(tc.tile_pool(name="ps_lo", bufs=1, space="PSUM"))
        ps_wl = ctx.enter_context(tc.tile_pool(name="ps_wl", bufs=1, space="PSUM"))
        ps_dl = ctx.enter_context(tc.tile_pool(name="ps_dl", bufs=2, space="PSUM"))

        # ---- constants / replicated params ----
        ident = const.tile([128, 128], F32)
        make_identity(nc, ident)
        ident_bf = const.tile([128, 128], BF16)
        nc.vector.tensor_copy(out=ident_bf, in_=ident)

        w1_sb = const.tile([128, HC, RH], F32)
        nc.gpsimd.dma_start(out=w1_sb, in_=w1_d.rearrange("(c p) m -> p c m", p=128))
        w1_r = const.tile([128, HC, RH], F32R)
        nc.vector.tensor_copy(out=w1_r, in_=w1_sb)
        b1_sb = const.tile([128, RC], F32)
        nc.gpsimd.dma_start(out=b1_sb, in_=b1_d.rearrange("(c p) -> p c", p=128))
        w2_sb = const.tile([128, RC, E], F32)
        nc.gpsimd.dma_start(out=w2_sb, in_=w2_d.rearrange("(c p) e -> p c e", p=128))
        # b2 replicated to all partitions (added on DVE, not via PE)
        b2_rep = const.tile([128, E], F32)
        nc.gpsimd.dma_start(
            out=b2_rep, in_=b2_d[:].unsqueeze(0).to_broadcast((128, E))
        )
        # LoRA params concatenated over (e, r): index m = e*R + r.
        acat_sb = const.tile([128, HC, E, R], F32)
        for e in range(E):
            for c in range(HC):
                nc.gpsimd.dma_start(
                    out=acat_sb[:, c, e, :],
                    in_=a_d[e, c * 128 : (c + 1) * 128, :],
                )
        acat_r = const.tile([128, HC, E, R], F32R)
        nc.vector.tensor_copy(out=acat_r, in_=acat_sb)
        bcat_sb = const.tile([ER, H], F32)
        for e in range(E):
            nc.gpsimd.dma_start(out=bcat_sb[e * R : (e + 1) * R, :], in_=bm_d[e, :, :])
        # LoRA SCALING (=2.0) folded into Bcat here; delta path is bf16.
        bcat_bf = const.tile([ER, H], BF16)
        nc.vector.tensor_scalar(
            out=bcat_bf, in0=bcat_sb, scalar1=float(SCALING), scalar2=None,
            op0=ALU.mult,
        )

        if dummy_d is not None:
            dnm = const.tile([1, 4], F32)
            nc.vector.memset(dnm, 1.0)
            nc.sync.dma_start(out=dummy_d[:, :], in_=dnm)

        loop_ctx = tc.For_i(0, niter, 1) if niter > 1 else None
        if loop_ctx is not None:
            ctx.enter_context(loop_ctx)

        def emit_front(i):
            """loads + xT transposes (exact fp32) + f32r copy, c-chunk rotated"""
            tok = i * TT
            x_nat = xin_p.tile([128, JJ, H], F32)
            nc.sync.dma_start(
                out=x_nat,
                in_=x_d[tok : tok + TT, :].rearrange("(j p) h -> p j h", p=128),
            )
            bo = bout_p.tile([128, JJ, H], F32)
            nc.scalar.dma_start(
                out=bo,
                in_=base_d[tok : tok + TT, :].rearrange("(j p) h -> p j h", p=128),
            )
            xt_r = xt_p.tile([128, HC, TT], F32R)
            for c in range(HC):
                xtp = ps_xtp.tile([128, TT], F32, tag="xtp")
                for tj in range(JJ):
                    nc.tensor.transpose(
                        out=xtp[:, ts(tj, 128)],
                        in_=x_nat[:, tj, ts(c, 128)],
                        identity=ident,
                    )
                nc.scalar.copy(out=xt_r[:, c, :], in_=xtp)
            return {"bo": bo, "xt_r": xt_r, "tok": tok}

        def emit_router(st):
            """mm1 -> silu -> (low interleaved) -> mm2"""
            xt_r = st["xt_r"]
            h_ps = ps_h.tile([128, RC, TT], F32, tag="h")
            for c2 in range(RC):
                for c in range(HC):
                    nc.tensor.matmul(
                        out=h_ps[:, c2, :],
                        lhsT=w1_r[:, c, ts(c2, 128)],
                        rhs=xt_r[:, c, :],
                        start=(c == 0),
                        stop=(c == HC - 1),
                    )
            # silu(z) = z * sigmoid(z), z = h + b1: ACT computes sigmoid and
            # z (identity+bias), Pool multiplies (keeps DVE free)
            sg_sb = ht_p.tile([128, RC, TT], F32, tag="sg")
            hb_sb = ht_p.tile([128, RC, TT], F32, tag="hb")
            ht_sb = ht_p.tile([128, RC, TT], F32, tag="ht")
            for c2 in range(RC):
                nc.scalar.activation(
                    out=sg_sb[:, c2, :], in_=h_ps[:, c2, :],
                    func=AF.Sigmoid, bias=b1_sb[:, c2 : c2 + 1],
                )
                nc.scalar.activation(
                    out=hb_sb[:, c2, :], in_=h_ps[:, c2, :],
                    func=AF.Identity, bias=b1_sb[:, c2 : c2 + 1],
                )
            nc.gpsimd.tensor_tensor(out=ht_sb, in0=hb_sb, in1=sg_sb, op=ALU.mult)

            # lowT[(e,r), t] = (x @ Acat)^T (f32r, N=512)
            low_ps = ps_lo.tile([ER, TT], F32, tag="lo")
            for c in range(HC):
                nc.tensor.matmul(
                    out=low_ps,
                    lhsT=acat_r[:, c, :, :],
                    rhs=xt_r[:, c, :],
                    start=(c == 0),
                    stop=(c == HC - 1),
                )
            # wl bank: lg f32 in [:, 0:20], wrt bf16 in f32-cols [128:384]
            wl = ps_wl.tile([128, 512], F32, tag="wl")
            lg_ps = wl[:, 0 : JJ * E].rearrange("p (j e) -> p j e", j=JJ)
            # router mm2 (token-major logits), exact fp32, b2 added on DVE
            for j in range(JJ):
                for c2 in range(RC):
                    nc.tensor.matmul(
                        out=lg_ps[:, j, :],
                        lhsT=ht_sb[:, c2, ts(j, 128)],
                        rhs=w2_sb[:, c2, :],
                        start=(c2 == 0),
                        stop=(c2 == RC - 1),
                    )
            st["low_ps"], st["wl"], st["lg_ps"] = low_ps, wl, lg_ps

        def emit_weights(st):
            """j-merged top-2 + renormalized weights:
            w[e] = sigmoid(2*lg_e - m1 - m2) * [lg_e >= m2], expanded over r."""
            lg = small_p.tile([128, JJ, E], F32, tag="lg")
            nc.vector.tensor_tensor(
                out=lg, in0=st["lg_ps"],
                in1=b2_rep.unsqueeze(1).to_broadcast((128, JJ, E)), op=ALU.add,
            )
            m1 = small_p.tile([128, JJ], F32, tag="m1")
            nc.vector.tensor_reduce(out=m1, in_=lg, axis=AX.X, op=ALU.max)
            mask1 = small_p.tile([128, JJ, E], F32, tag="mask1")
            nc.vector.tensor_tensor(
                out=mask1, in0=lg, in1=m1.unsqueeze(2).to_broadcast((128, JJ, E)),
                op=ALU.is_equal,
            )
            masked = small_p.tile([128, JJ, E], F32, tag="masked")
            nc.vector.scalar_tensor_tensor(
                out=masked, in0=mask1, scalar=-1e30, in1=lg,
                op0=ALU.mult, op1=ALU.add,
            )
            m2 = small_p.tile([128, JJ], F32, tag="m2")
            nc.vector.tensor_reduce(out=m2, in_=masked, axis=AX.X, op=ALU.max)
            s2 = small_p.tile([128, JJ], F32, tag="s2")
            nc.vector.tensor_tensor(out=s2, in0=m1, in1=m2, op=ALU.add)
            argt = small_p.tile([128, JJ, E], F32, tag="argt")
            nc.vector.scalar_tensor_tensor(
                out=argt, in0=lg, scalar=2.0,
                in1=s2.unsqueeze(2).to_broadcast((128, JJ, E)),
                op0=ALU.mult, op1=ALU.subtract,
            )
            sig = small_p.tile([128, JJ, E], F32, tag="sig")
            nc.scalar.activation(out=sig, in_=argt, func=AF.Sigmoid)
            mge = small_p.tile([128, JJ, E], F32, tag="mge")
            nc.vector.tensor_tensor(
                out=mge, in0=lg, in1=m2.unsqueeze(2).to_broadcast((128, JJ, E)),
                op=ALU.is_ge,
            )
            # fused weight + expansion over r: w_exp[t, j, e, r] = sig*mge
            w_exp = small_p.tile([128, JJ, E, R], BF16, tag="w_exp")
            nc.vector.tensor_tensor(
                out=w_exp,
                in0=sig.unsqueeze(3).to_broadcast((128, JJ, E, R)),
                in1=mge.unsqueeze(3).to_broadcast((128, JJ, E, R)),
                op=ALU.mult,
            )
            st["w_exp"] = w_exp

        def emit_m(st):
            """wT transpose (into wl bank, bf16) + weighted-low"""
            # bf16 view of wl f32-cols [128:384] = 512 bf16 cols
            wrt_ps = st["wl"][:, 128:384].bitcast(BF16)[0:ER, :].rearrange(
                "p (j t) -> p j t", j=JJ
            )
            for j in range(JJ):
                nc.tensor.transpose(
                    out=wrt_ps[:, j, :],
                    in_=st["w_exp"][:, j, :, :].rearrange("p e r -> p (e r)"),
                    identity=ident_bf,
                )
            wrt_sb = lw_p.tile([ER, JJ, 128], BF16, tag="wrt_sb")
            nc.scalar.copy(out=wrt_sb, in_=wrt_ps)
            lw_sb = lw_p.tile([ER, TT], BF16)
            nc.vector.tensor_tensor(
                out=lw_sb,
                in0=st["low_ps"],
                in1=wrt_sb.rearrange("p j t -> p (j t)"),
                op=ALU.mult,
            )
            st["lw_sb"] = lw_sb

        def emit_b(st):
            """back half: delta matmuls (bf16) in 5 bank-aligned chunks of the
            flat (j, h) output, fused PSUM+base adds, store"""
            bo, lw_sb, tok = st["bo"], st["lw_sb"], st["tok"]
            bo_flat = bo.rearrange("p j h -> p (j h)")
            for c5, parts in enumerate(DELTA_CHUNKS):
                dl = ps_dl.tile([128, 512], F32, tag="dl")
                for j, h0, h1, off in parts:
                    nc.tensor.matmul(
                        out=dl[:, off : off + (h1 - h0)],
                        lhsT=lw_sb[:, ts(j, 128)],
                        rhs=bcat_bf[:, h0:h1],
                        start=True, stop=True,
                    )
                nc.vector.tensor_tensor(
                    out=bo_flat[:, c5 * 512 : (c5 + 1) * 512],
                    in0=dl,
                    in1=bo_flat[:, c5 * 512 : (c5 + 1) * 512],
                    op=ALU.add,
                )
                if c5 == 2:
                    # tokens j0/j1 fully accumulated -> store first half early
                    nc.gpsimd.dma_start(
                        out=out_d[tok : tok + 256, :].rearrange(
                            "(j p) h -> p j h", p=128
                        ),
                        in_=bo[:, 0:2, :],
                    )
            nc.gpsimd.dma_start(
                out=out_d[tok + 256 : tok + TT, :].rearrange("(j p) h -> p j h", p=128),
                in_=bo[:, 2:4, :],
            )

        prev = None
        for i in range(ntiles):
            st = emit_front(i)
            emit_router(st)
            emit_weights(st)
            emit_m(st)
            if prev is not None:
                emit_b(prev)
            prev = st
        emit_b(prev)

    return nc


_CACHE = {}


def _get_nc(t_core=T_CORE, niter=1, timing_mode=False):
    key = (t_core, niter, timing_mode)
    if key not in _CACHE:
        nc = build_kernel(t_core, niter, timing_mode)
        nc.finalize()
        _CACHE[key] = nc
    return _CACHE[key]


def kernel(x, base_output, W1, b1, W2, b2, A, Bm):
    x = np.ascontiguousarray(np.asarray(x), dtype=np.float32)
    base_output = np.ascontiguousarray(np.asarray(base_output), dtype=np.float32)
    W1 = np.ascontiguousarray(np.asarray(W1), dtype=np.float32)
    b1 = np.ascontiguousarray(np.asarray(b1), dtype=np.float32)
    W2 = np.ascontiguousarray(np.asarray(W2), dtype=np.float32)
    b2 = np.ascontiguousarray(np.asarray(b2), dtype=np.float32)
    A = np.ascontiguousarray(np.asarray(A), dtype=np.float32)
    Bm = np.ascontiguousarray(np.asarray(Bm), dtype=np.float32)

    B, S, _ = x.shape
    assert B * S == N_CORES * T_CORE
    xs = x.reshape(N_CORES, T_CORE, H)
    bs = base_output.reshape(N_CORES, T_CORE, H)

    nc = _get_nc()
    in_maps = [
        {
            "x": np.ascontiguousarray(xs[i]),
            "base": np.ascontiguousarray(bs[i]),
            "W1": W1, "b1": b1, "W2": W2, "b2": b2, "A": A, "Bm": Bm,
        }
        for i in range(N_CORES)
    ]
    res = run_bass_kernel_spmd(nc, in_maps, list(range(N_CORES))).results
    out = np.stack([res[i]["out"] for i in range(N_CORES)], axis=0)
    return out.reshape(B, S, H).astype(np.float32)
